# revision 1
# baseline (speedup 1.0000x reference)
"""Trainium2 Bass kernel for nn_BidirectionalTrustModel.

Computes, for each of N=65536 independent observation sequences:
  1. A sequential scan over T=64 steps updating a per-sequence trust
     interval [low, high] for 2 capability dims (sens, proc).
  2. trust = prod over dims of (sum_j d_j * m_j / sum_j m_j) where
     m is the 10-bin interval mask and d_j = (1+exp(beta*(req-s_j)))^(-zeta^2).
     (The reference's 10x10 outer-product normalization factorizes exactly.)

Sharding: pure data-parallel over N across 8 NeuronCores (8192 seqs/core).

Device algorithm (exact fp32 emulation of the reference scan):
  per step, with precomputed planes A = succ ? cap : 0, B = fail ? cap : 2:
    g1 = A > high ; A' = A - 4*g1          (guard: succ above the interval)
    lo2 = min(max(low, A'), B)
    hs  = max(high, A)
    g2 = B < low  ; B' = B + 4*g2          (guard: fail below the interval)
    hi1 = min(hs, B')
    eq = (lo2 == hi1) ; low' = lo2 - 0.1*eq ; high' = hi1
  This reproduces the reference's branch/fixup semantics bit-exactly in fp32:
  succ and fail are mutually exclusive, high is never 0 (the high2==0 fixup
  branch is dead), and the A-neutral 0 can only lift low from -0.05 to 0,
  which is bisimilar (all caps are >= 0.05; identical masks and compares).
"""

import numpy as np

BINS = 10
T = 64
N_TOTAL = 65536
N_CORES = 8
P = 128                 # SBUF partitions
NC = N_TOTAL // N_CORES  # 8192 sequences per core
K = NC // P             # 64 free-dim columns per dim
F4 = T * K              # 4096 columns for [T, NC] planes laid out [P, T*K]

_F32 = np.float32
STEPS = ((np.arange(BINS, dtype=np.float32) + _F32(0.5)) * _F32(0.1)).astype(np.float32)

# columns (of 128 state columns) scanned on the vector engine; the rest
# go to GpSimd. 128 = everything on DVE (GpSimd's ~156ns fixed per-op cost
# loses on the serial chain of small ops; it does the precompute instead).
SCAN_DVE_COLS = 128
# t-chunk sizes for DMA + precompute pipelining (small first chunks let the
# DVE scan start early)
CHUNK_STEPS = [2, 2, 4, 8, 8, 8, 8, 8, 8, 8]
assert sum(CHUNK_STEPS) == T

_NC_CACHE = {}


def _build_nc():
    import concourse.bass as bass
    import concourse.mybir as mybir
    import concourse.tile as tile
    from concourse.tile import ScopedClock

    dt = mybir.dt
    Alu = mybir.AluOpType
    Act = mybir.ActivationFunctionType

    class PatchedTileContext(tile.TileContext):
        """This walrus build only lowers ONE sem wait per SP Drain; split the
        tail drain's waits across extra drain instructions."""
        MAX_WAITS = 1

        def _drain_and_barrier(self, tick_clock, wait_clock):
            nc = self.nc
            drain_inst = nc.sync.drain()
            wait_clock.add_sem_waits(
                drain_inst.ins, ScopedClock({None: tick_clock.global_clock})
            )
            si = drain_inst.ins.sync_info
            kmax = self.MAX_WAITS
            if si is not None and si.on_wait and len(si.on_wait) > kmax:
                waits = list(si.on_wait)
                drain_inst.ins.sync_info = mybir.SyncInfo(
                    on_wait=waits[:kmax], on_update=list(si.on_update)
                )
                rest = waits[kmax:]
                for i in range(0, len(rest), kmax):
                    extra = nc.sync.drain()
                    extra.ins.sync_info = mybir.SyncInfo(
                        on_wait=rest[i : i + kmax], on_update=[]
                    )
            nc.all_engine_barrier()
            assert self.sems is not None
            popped = nc._tile_sem_poison_stack.pop()
            assert popped is self._sem_poison
            nc.clear_and_free_semaphores(list(self.sems.allocated().values()))
            nc.all_engine_barrier()

    def _split_sync_waits(nc):
        """This walrus build lowers at most ONE sync wait per instruction.
        Move extra waits onto same-engine NoOps inserted just before."""
        n_split = 0
        for f in nc.m.functions:
            for bb in f.blocks:
                il = bb.instructions
                new = []
                for ins in il:
                    si = ins.sync_info
                    if si is not None and si.on_wait and len(si.on_wait) > 1:
                        waits = list(si.on_wait)
                        for w in waits[:-1]:
                            nop = mybir.InstNoOp(name=f"I-wsplit-{nc.next_id()}")
                            nop.engine = ins.engine
                            nop.sync_info = mybir.SyncInfo(on_wait=[w], on_update=[])
                            nc.register_instruction(nop, overwrite=True)
                            new.append(nop)
                            n_split += 1
                        ins.sync_info = mybir.SyncInfo(
                            on_wait=[waits[-1]], on_update=list(si.on_update)
                        )
                    new.append(ins)
                il[:] = new
        return n_split

    nc = bass.Bass(target_bir_lowering=False, trn_type="TRN2")

    f32, i32 = dt.float32, dt.int32
    obs_s_d = nc.declare_dram_parameter("obs_s", [P, F4], f32, isOutput=False)
    obs_p_d = nc.declare_dram_parameter("obs_p", [P, F4], f32, isOutput=False)
    p0_d = nc.declare_dram_parameter("p0", [P, F4], i32, isOutput=False)
    p1_d = nc.declare_dram_parameter("p1", [P, F4], i32, isOutput=False)
    bt_d = nc.declare_dram_parameter("bt", [P, K * BINS], f32, isOutput=False)
    reqb_s_d = nc.declare_dram_parameter("reqb_s", [P, K * BINS], f32, isOutput=False)
    reqb_p_d = nc.declare_dram_parameter("reqb_p", [P, K * BINS], f32, isOutput=False)
    bz_d = nc.declare_dram_parameter("bz", [P, 4], f32, isOutput=False)
    out_d = nc.declare_dram_parameter("trust", [P, K], f32, isOutput=True)

    with PatchedTileContext(nc) as tc:
        with tc.tile_pool(name="planes", bufs=1) as planes:
            # Per-chunk A/B event plane tiles (separate tiles keep Tile's
            # dependency tracking precise so the DVE scan of chunk c only
            # waits on chunk c's precompute).
            # Within a chunk, cols = t_local*128 + dim*64 + k.
            # A = cap on success else 0 (0 is a safe neutral for the low-side
            # max: lifting low from -0.05 to 0 is bisimilar).
            # B = cap on failure else 2.0 (neutral for the min / is_lt guard).
            NCHUNK = len(CHUNK_STEPS)
            CHUNK_T0 = [sum(CHUNK_STEPS[:c]) for c in range(NCHUNK)]
            A_chunks = [planes.tile([P, CHUNK_STEPS[c] * 2 * K], f32,
                                    tag=f"A{c}", name=f"Ach{c}")
                        for c in range(NCHUNK)]
            B_chunks = [planes.tile([P, CHUNK_STEPS[c] * 2 * K], f32,
                                    tag=f"B{c}", name=f"Bch{c}")
                        for c in range(NCHUNK)]

            # Keep every pool open for the whole kernel: closing a pool lets
            # the stack allocator hand its SBUF range to the next pool, and
            # Tile then serializes the new pool's writers behind ALL of the
            # old pool's accessors (released-zone overlap hazard) — which
            # destroys the DMA/precompute/scan pipeline.
            with tc.tile_pool(name="stage", bufs=3) as stage, \
                 tc.tile_pool(name="state", bufs=1) as state_pool, \
                 tc.tile_pool(name="scantmp", bufs=2) as stp, \
                 tc.tile_pool(name="final", bufs=1) as fin:
                CHMAX = max(CHUNK_STEPS) * K
                for c in range(NCHUNK):
                    CH = CHUNK_STEPS[c] * K
                    obs_s = stage.tile([P, CHMAX], f32, tag="obs_s", name=f"obs_s{c}")[:, :CH]
                    obs_p = stage.tile([P, CHMAX], f32, tag="obs_p", name=f"obs_p{c}")[:, :CH]
                    p0 = stage.tile([P, CHMAX], i32, tag="p0", name=f"p0_{c}")[:, :CH]
                    p1 = stage.tile([P, CHMAX], i32, tag="p1", name=f"p1_{c}")[:, :CH]
                    sB = stage.tile([P, CHMAX], f32, tag="sB", name=f"sB{c}")[:, :CH]
                    tB = stage.tile([P, CHMAX], f32, tag="tB", name=f"tB{c}")[:, :CH]
                    sl = slice(CHUNK_T0[c] * K, CHUNK_T0[c] * K + CH)
                    nc.sync.dma_start(obs_s[:], obs_s_d[:, sl])
                    nc.sync.dma_start(obs_p[:], obs_p_d[:, sl])
                    nc.sync.dma_start(p0[:], p0_d[:, sl])
                    nc.sync.dma_start(p1[:], p1_d[:, sl])

                    def tk(ap):  # [P, TCH, K] view of a chunk staging plane
                        return ap[:].rearrange("p (t k) -> p t k", k=K)

                    def abview(ap, dim):  # destination view inside a chunk tile
                        v = ap[:].rearrange("p (t w) -> p t w", w=2 * K)
                        return v[:, :, dim * K : (dim + 1) * K]

                    # B = obs*p0 + (2 - 2*p0): exact (cap+0 / 0+2).
                    # Chunk 0 precomputes on the DVE itself (1.4us) instead of
                    # waiting on GpSimd's slower first chunk (~3us lead-in).
                    peng = nc.vector if c == 0 else nc.gpsimd
                    peng.tensor_scalar(
                        sB[:], p0[:], -2.0, 2.0, Alu.mult, Alu.add)
                    for dim, obs in ((0, obs_s), (1, obs_p)):
                        peng.tensor_tensor(
                            abview(A_chunks[c], dim), tk(obs), tk(p1),
                            Alu.mult)
                        peng.tensor_tensor(
                            tB[:], obs[:], p0[:], Alu.mult)
                        peng.tensor_tensor(
                            abview(B_chunks[c], dim), tk(tB), tk(sB),
                            Alu.add)

                # ---- d-weights (independent of the scan; emitted first so
                # the ACT engine computes them while the scan runs) ----
                KB = K * BINS  # 640
                bt = fin.tile([P, KB], f32, tag="bt")
                reqb_s = fin.tile([P, KB], f32, tag="reqb_s")
                reqb_p = fin.tile([P, KB], f32, tag="reqb_p")
                bz = fin.tile([P, 4], f32, tag="bz")
                nc.sync.dma_start(bt[:], bt_d[:, :])
                nc.sync.dma_start(reqb_s[:], reqb_s_d[:, :])
                nc.sync.dma_start(reqb_p[:], reqb_p_d[:, :])
                nc.sync.dma_start(bz[:], bz_d[:, :])

                nzz = fin.tile([P, 2], f32, tag="nzz")
                nc.gpsimd.tensor_tensor(nzz[:], bz[:, 2:4], bz[:, 2:4], Alu.mult)
                nc.gpsimd.tensor_scalar(nzz[:], nzz[:], -1.0, None, Alu.mult)

                d_tiles = []
                for dim, reqb in ((0, reqb_s), (1, reqb_p)):
                    t1 = fin.tile([P, KB], f32, tag=f"t1_{dim}")
                    sp = fin.tile([P, KB], f32, tag=f"sp_{dim}")
                    dti = fin.tile([P, KB], f32, tag=f"d_{dim}")
                    # d = exp(-zeta^2 * ln(1 + exp(beta * (req - s))))
                    nc.gpsimd.tensor_tensor(t1[:], reqb[:], bt[:], Alu.subtract)
                    nc.scalar.activation(sp[:], t1[:], Act.Exp,
                                         scale=bz[:, dim : dim + 1])
                    nc.gpsimd.tensor_scalar(t1[:], sp[:], 1.0, None, Alu.add)
                    nc.scalar.activation(sp[:], t1[:], Act.Ln)
                    nc.scalar.activation(dti[:], sp[:], Act.Exp,
                                         scale=nzz[:, dim : dim + 1])
                    d_tiles.append(dti)

                W = 2 * K  # 128: sens || proc
                # column split: DVE scans cols [0:WD], GpSimd cols [WD:W].
                # Separate per-engine state tiles avoid false tile-level deps.
                WD = SCAN_DVE_COLS
                engines = [(nc.vector, 0, WD, "d")]
                if WD < W:
                    engines.append((nc.gpsimd, WD, W, "g"))
                state = {}
                for eng, c0, c1, sfx in engines:
                    Wx = c1 - c0
                    lo_t = state_pool.tile([P, Wx], f32, tag=f"low{sfx}")
                    hi_t = state_pool.tile([P, Wx], f32, tag=f"high{sfx}")
                    eng.memset(lo_t[:], 0.0)
                    eng.memset(hi_t[:], 1.0)
                    state[sfx] = (lo_t, hi_t)

                t_to_chunk = []
                for c in range(NCHUNK):
                    t_to_chunk += [(c, i) for i in range(CHUNK_STEPS[c])]
                for t in range(T):
                    tc_idx, tl_idx = t_to_chunk[t]
                    for eng, c0, c1, sfx in engines:
                        Wx = c1 - c0
                        A = A_chunks[tc_idx][:, tl_idx * W + c0 : tl_idx * W + c1]
                        B = B_chunks[tc_idx][:, tl_idx * W + c0 : tl_idx * W + c1]
                        lo = state[sfx][0][:]
                        hi = state[sfx][1][:]
                        g1 = stp.tile([P, Wx], f32, tag=f"g1{sfx}")
                        Ap = stp.tile([P, Wx], f32, tag=f"Ap{sfx}")
                        lo1 = stp.tile([P, Wx], f32, tag=f"lo1{sfx}")
                        lo2 = stp.tile([P, Wx], f32, tag=f"lo2{sfx}")
                        hs = stp.tile([P, Wx], f32, tag=f"hs{sfx}")
                        g2 = stp.tile([P, Wx], f32, tag=f"g2{sfx}")
                        Bp = stp.tile([P, Wx], f32, tag=f"Bp{sfx}")
                        eq = stp.tile([P, Wx], f32, tag=f"eq{sfx}")

                        # ordering interleaves the succ/fail sub-chains so
                        # dependent ops are not back-to-back (hides the DVE
                        # write-ack latency between dependent instructions)
                        eng.tensor_tensor(g1[:], A, hi, Alu.is_gt)
                        eng.tensor_tensor(g2[:], B, lo, Alu.is_lt)
                        eng.tensor_tensor(hs[:], hi, A, Alu.max)
                        # succ: Ap = A - 4*(A > high); fail: Bp = B + 4*(B < low)
                        eng.scalar_tensor_tensor(
                            Ap[:], g1[:], -4.0, A, Alu.mult, Alu.add)
                        eng.scalar_tensor_tensor(
                            Bp[:], g2[:], 4.0, B, Alu.mult, Alu.add)
                        eng.tensor_tensor(lo1[:], lo, Ap[:], Alu.max)
                        eng.tensor_tensor(hi, hs[:], Bp[:], Alu.min)
                        eng.tensor_tensor(lo2[:], lo1[:], B, Alu.min)
                        # fixup: low' = lo2 - 0.1*(lo2 == high')
                        eng.tensor_tensor(eq[:], lo2[:], hi, Alu.is_equal)
                        eng.scalar_tensor_tensor(
                            lo, eq[:], -0.1, lo2[:], Alu.mult, Alu.add)

                if WD == W:
                    # single-engine state is already canonical [P, W]
                    low, high = state["d"][0], state["d"][1]
                else:
                    # repack split state into canonical [P, W] (cols = dim*K + k)
                    lowC = state_pool.tile([P, W], f32, tag="lowC")
                    highC = state_pool.tile([P, W], f32, tag="highC")
                    for eng, c0, c1, sfx in engines:
                        nc.vector.tensor_copy(lowC[:, c0:c1], state[sfx][0][:])
                        nc.vector.tensor_copy(highC[:, c0:c1], state[sfx][1][:])
                    low, high = lowC, highC

                # ---- final phase (tail after the scan) ----
                bt3 = bt[:].rearrange("p (k j) -> p k j", j=BINS)
                UC = []
                for dim, meng in ((0, nc.vector), (1, nc.vector)):
                    lowv = low[:, dim * K : (dim + 1) * K]
                    highv = high[:, dim * K : (dim + 1) * K]
                    lowb = lowv.unsqueeze(2).broadcast_to((P, K, BINS))
                    highb = highv.unsqueeze(2).broadcast_to((P, K, BINS))
                    m1 = fin.tile([P, KB], f32, tag=f"m1_{dim}")
                    m = fin.tile([P, KB], f32, tag=f"m_{dim}")
                    dm = fin.tile([P, KB], f32, tag=f"dm_{dim}")

                    # m = (s >= low) & (s <= high); one dim's mask chain on
                    # GpSimd, the other on DVE, in parallel
                    meng.tensor_tensor(m1[:].rearrange("p (k j) -> p k j", j=BINS),
                                       bt3, lowb, Alu.is_ge)
                    meng.tensor_tensor(m[:].rearrange("p (k j) -> p k j", j=BINS),
                                       bt3, highb, Alu.is_le)
                    meng.tensor_tensor(m[:], m[:], m1[:], Alu.mult)
                    nc.vector.tensor_tensor(dm[:], d_tiles[dim][:], m[:], Alu.mult)

                    U = fin.tile([P, K], f32, tag=f"U_{dim}")
                    C = fin.tile([P, K], f32, tag=f"C_{dim}")
                    nc.vector.tensor_reduce(
                        U[:], dm[:].rearrange("p (k j) -> p k j", j=BINS),
                        mybir.AxisListType.X, Alu.add)
                    nc.vector.tensor_reduce(
                        C[:], m[:].rearrange("p (k j) -> p k j", j=BINS),
                        mybir.AxisListType.X, Alu.add)
                    UC.append((U, C))

                # trust = (U0*U1) / (C0*C1)
                uu = fin.tile([P, K], f32, tag="uu")
                cc = fin.tile([P, K], f32, tag="cc")
                rr = fin.tile([P, K], f32, tag="rr")
                tr = fin.tile([P, K], f32, tag="tr")
                nc.vector.tensor_tensor(uu[:], UC[0][0][:], UC[1][0][:], Alu.mult)
                nc.vector.tensor_tensor(cc[:], UC[0][1][:], UC[1][1][:], Alu.mult)
                nc.vector.reciprocal(rr[:], cc[:])
                nc.vector.tensor_tensor(tr[:], uu[:], rr[:], Alu.mult)
                nc.sync.dma_start(out_d[:, :], tr[:])

    _split_sync_waits(nc)
    return nc


def _get_nc():
    if "nc" not in _NC_CACHE:
        _NC_CACHE["nc"] = _build_nc()
    return _NC_CACHE["nc"]


def _marshal_core(inputs, c):
    """Build the per-core input map (pure slicing/layout, no arithmetic)."""
    n0, n1 = c * NC, (c + 1) * NC

    def lay(x):  # [T, NC] -> [P, T*K] with col = t*K + k, seq n = p*K + k
        return np.ascontiguousarray(
            x.reshape(T, P, K).transpose(1, 0, 2).reshape(P, F4))

    obs_s = lay(np.asarray(inputs["obs_task_sens_cap_seq"][:, n0:n1], dtype=np.float32))
    obs_p = lay(np.asarray(inputs["obs_task_proc_cap_seq"][:, n0:n1], dtype=np.float32))
    perf = np.asarray(inputs["inptasksperf"][:, n0:n1, :])
    p0 = lay(np.ascontiguousarray(perf[:, :, 0]).astype(np.int32))
    p1 = lay(np.ascontiguousarray(perf[:, :, 1]).astype(np.int32))

    def layreq(x):  # [NC] -> [P, K*BINS] broadcast each seq over 10 bins
        r = x.reshape(P, K, 1)
        return np.ascontiguousarray(np.broadcast_to(r, (P, K, BINS)).reshape(P, K * BINS))

    req_s = layreq(np.asarray(inputs["pred_task_sens_cap"][n0:n1, 0], dtype=np.float32))
    req_p = layreq(np.asarray(inputs["pred_task_proc_cap"][n0:n1, 0], dtype=np.float32))
    bt = np.ascontiguousarray(np.broadcast_to(np.tile(STEPS, K), (P, K * BINS))).astype(np.float32)
    betas = np.asarray(inputs["betas"], dtype=np.float32)
    zetas = np.asarray(inputs["zetas"], dtype=np.float32)
    bz = np.ascontiguousarray(
        np.broadcast_to(np.concatenate([betas, zetas]).astype(np.float32), (P, 4)))
    return {
        "obs_s": obs_s, "obs_p": obs_p, "p0": p0, "p1": p1,
        "bt": bt, "reqb_s": req_s, "reqb_p": req_p, "bz": bz,
    }


def kernel(**inputs) -> np.ndarray:
    from concourse.bass_utils import run_bass_kernel_spmd

    nc = _get_nc()
    in_maps = [_marshal_core(inputs, c) for c in range(N_CORES)]
    res = run_bass_kernel_spmd(nc, in_maps, core_ids=list(range(N_CORES)))
    out = np.empty((N_TOTAL, 1), dtype=np.float32)
    for c in range(N_CORES):
        out[c * NC : (c + 1) * NC, 0] = res.results[c]["trust"].reshape(NC)
    return out


# ---------------------------------------------------------------------------
# numpy mirror of the device algorithm (for validation only)
def _numpy_mirror(inputs):
    obs_s = np.asarray(inputs["obs_task_sens_cap_seq"], dtype=np.float32)
    obs_p = np.asarray(inputs["obs_task_proc_cap_seq"], dtype=np.float32)
    perf = np.asarray(inputs["inptasksperf"])
    p0 = (perf[:, :, 0] != 0)
    p1 = (perf[:, :, 1] != 0)
    betas = np.asarray(inputs["betas"], dtype=np.float32)
    zetas = np.asarray(inputs["zetas"], dtype=np.float32)
    req = [np.asarray(inputs["pred_task_sens_cap"][:, 0], dtype=np.float32),
           np.asarray(inputs["pred_task_proc_cap"][:, 0], dtype=np.float32)]
    N = obs_s.shape[1]
    trust = np.ones(N, dtype=np.float32)
    for dim, obs in ((0, obs_s), (1, obs_p)):
        low = np.zeros(N, np.float32)
        high = np.ones(N, np.float32)
        two = np.float32(2.0)
        inv = np.float32(0.1)
        four = np.float32(4.0)
        for t in range(T):
            A = np.where(p1[t], obs[t], np.float32(0.0)).astype(np.float32)
            B = np.where(p0[t], obs[t], two).astype(np.float32)
            g1 = (A > high).astype(np.float32)
            hs = np.maximum(high, A)
            Ap = ((-four) * g1 + A).astype(np.float32)
            lo1 = np.maximum(low, Ap)
            lo2 = np.minimum(lo1, B)
            g2 = (B < low).astype(np.float32)
            Bp = (four * g2 + B).astype(np.float32)
            high = np.minimum(hs, Bp)
            eqm = (lo2 == high).astype(np.float32)
            low = ((-inv) * eqm + lo2).astype(np.float32)
        m = ((STEPS[None, :] >= low[:, None]) & (STEPS[None, :] <= high[:, None]))
        z2 = np.float32(zetas[dim]) * np.float32(zetas[dim])
        p = np.float32(betas[dim]) * (req[dim][:, None] - STEPS[None, :])
        d = np.exp(-z2 * np.log1p(np.exp(p.astype(np.float64))))
        u = (d * m).sum(1) / m.sum(1)
        trust = trust * u.astype(np.float32)
    return trust[:, None]



# revision 13
# speedup vs baseline: 1.2032x; 1.2032x over previous
"""Trainium2 Bass kernel for nn_BidirectionalTrustModel.

Computes, for each of N=65536 independent observation sequences:
  1. A sequential scan over T=64 steps updating a per-sequence trust
     interval [low, high] for 2 capability dims (sens, proc).
  2. trust = prod over dims of (sum_j d_j * m_j / sum_j m_j) where
     m is the 10-bin interval mask and d_j = (1+exp(beta*(req-s_j)))^(-zeta^2).
     (The reference's 10x10 outer-product normalization factorizes exactly.)

Sharding: pure data-parallel over N across 8 NeuronCores (8192 seqs/core).

Device algorithm (exact emulation of the reference scan, in a x20-scaled
integer domain held in fp16 -- every value is an integer in [-82, 120],
exactly representable, so all compares/min/max/adds match the reference's
fp32 branch semantics bit-for-bit):
  per step, with input planes A = succ ? 20*cap : 0, B = fail ? 20*cap : 40,
  FX = per-event fixup constant (see below):
    g1 = A > high ; Ap = A - 80*g1     (guard: succ above the interval)
    lo1 = max(low, Ap) ; lo2 = min(lo1, B)
    hs  = max(high, A)
    g2 = B < low  ; Bp = B + 80*g2     (guard: fail below the interval)
    high' = min(hs, Bp)
    eq = (lo2 == high') ; low' = lo2 + FX*eq
  This reproduces the reference's branch/fixup semantics exactly:
  succ and fail are mutually exclusive, high is never 0 (the high2==0 fixup
  branch is dead), and the A-neutral 0 can only lift low from -1 to 0,
  which is bisimilar (all caps are >= 1; identical masks and compares).
  FX encodes the reference's fp32 rounding of (cap - 0.1f) relative to the
  grid point below: -2 when fp32 lands exactly on it, -1.5 when it rounds
  above, -2.5 when below. Fixup values only ever compare against grid
  integers, so the half-offsets replicate every fp32 comparison outcome
  (validated exhaustively on 200k random sequences).

fp16 gives the DVE's 2x perf mode on the 7 tensor_tensor ops per step
(the 3 scalar_tensor_tensor ops run at 1x), and host-packed A/B planes
halve the DMA volume vs shipping caps + perf bits separately.
"""

import numpy as np

BINS = 10
T = 64
N_TOTAL = 65536
N_CORES = 8
P = 128                 # SBUF partitions
NC = N_TOTAL // N_CORES  # 8192 sequences per core
K = NC // P             # 64 free-dim columns per dim
W = 2 * K               # 128 state columns: col = dim*K + k
FW = T * W              # 8192 columns for the [P, T*W] A/B planes

_F32 = np.float32
STEPS = ((np.arange(BINS, dtype=np.float32) + _F32(0.5)) * _F32(0.1)).astype(np.float32)

# Per-cap fixup constants for the x20 domain: the reference computes
# low = cap - 0.1f in fp32, which lands exactly on / above / below the grid
# point two units down depending on the bin. -2 / -1.5 / -2.5 replicate
# every comparison against grid values.
_FIX_OFF = np.empty(BINS, np.float32)
for _k in range(BINS):
    _v = np.float32(STEPS[_k] - np.float32(0.1))
    _below = STEPS[_k - 1] if _k > 0 else np.float32(0.0)
    _FIX_OFF[_k] = -2.0 if _v == _below else (-1.5 if _v > _below else -2.5)

# t-chunk sizes for DMA pipelining (small first chunks let the scan start
# early; A/B need no device-side precompute, so chunks feed the DVE directly)
CHUNK_STEPS = [2, 2, 4, 8, 8, 8, 8, 8, 8, 8]
assert sum(CHUNK_STEPS) == T

_NC_CACHE = {}


def _build_nc():
    import concourse.bass as bass
    import concourse.mybir as mybir
    import concourse.tile as tile
    from concourse.tile import ScopedClock

    dt = mybir.dt
    Alu = mybir.AluOpType
    Act = mybir.ActivationFunctionType

    class PatchedTileContext(tile.TileContext):
        """This walrus build only lowers ONE sem wait per SP Drain; split the
        tail drain's waits across extra drain instructions."""
        MAX_WAITS = 1

        def _drain_and_barrier(self, tick_clock, wait_clock):
            nc = self.nc
            drain_inst = nc.sync.drain()
            wait_clock.add_sem_waits(
                drain_inst.ins, ScopedClock({None: tick_clock.global_clock})
            )
            si = drain_inst.ins.sync_info
            kmax = self.MAX_WAITS
            if si is not None and si.on_wait and len(si.on_wait) > kmax:
                waits = list(si.on_wait)
                drain_inst.ins.sync_info = mybir.SyncInfo(
                    on_wait=waits[:kmax], on_update=list(si.on_update)
                )
                rest = waits[kmax:]
                for i in range(0, len(rest), kmax):
                    extra = nc.sync.drain()
                    extra.ins.sync_info = mybir.SyncInfo(
                        on_wait=rest[i : i + kmax], on_update=[]
                    )
            nc.all_engine_barrier()
            assert self.sems is not None
            popped = nc._tile_sem_poison_stack.pop()
            assert popped is self._sem_poison
            nc.clear_and_free_semaphores(list(self.sems.allocated().values()))
            nc.all_engine_barrier()

    def _split_sync_waits(nc):
        """This walrus build lowers at most ONE sync wait per instruction.
        Move extra waits onto same-engine NoOps inserted just before."""
        n_split = 0
        for f in nc.m.functions:
            for bb in f.blocks:
                il = bb.instructions
                new = []
                for ins in il:
                    si = ins.sync_info
                    if si is not None and si.on_wait and len(si.on_wait) > 1:
                        waits = list(si.on_wait)
                        for w in waits[:-1]:
                            nop = mybir.InstNoOp(name=f"I-wsplit-{nc.next_id()}")
                            nop.engine = ins.engine
                            nop.sync_info = mybir.SyncInfo(on_wait=[w], on_update=[])
                            nc.register_instruction(nop, overwrite=True)
                            new.append(nop)
                            n_split += 1
                        ins.sync_info = mybir.SyncInfo(
                            on_wait=[waits[-1]], on_update=list(si.on_update)
                        )
                    new.append(ins)
                il[:] = new
        return n_split

    nc = bass.Bass(target_bir_lowering=False, trn_type="TRN2")

    f32, f16 = dt.float32, dt.float16
    A_d = nc.declare_dram_parameter("Aplane", [P, FW], f16, isOutput=False)
    B_d = nc.declare_dram_parameter("Bplane", [P, FW], f16, isOutput=False)
    FX_d = nc.declare_dram_parameter("FXplane", [P, FW], f16, isOutput=False)
    bt_d = nc.declare_dram_parameter("bt", [P, K * BINS], f32, isOutput=False)
    bt20_d = nc.declare_dram_parameter("bt20", [P, K * BINS], f32, isOutput=False)
    reqb_s_d = nc.declare_dram_parameter("reqb_s", [P, K * BINS], f32, isOutput=False)
    reqb_p_d = nc.declare_dram_parameter("reqb_p", [P, K * BINS], f32, isOutput=False)
    bz_d = nc.declare_dram_parameter("bz", [P, 4], f32, isOutput=False)
    out_d = nc.declare_dram_parameter("trust", [P, K], f32, isOutput=True)

    with PatchedTileContext(nc) as tc:
        # Keep every pool open for the whole kernel: closing a pool lets
        # the stack allocator hand its SBUF range to the next pool, and
        # Tile then serializes the new pool's writers behind ALL of the
        # old pool's accessors (released-zone overlap hazard) -- which
        # destroys the DMA/scan pipeline.
        with tc.tile_pool(name="stage", bufs=3) as stage, \
             tc.tile_pool(name="state", bufs=1) as state_pool, \
             tc.tile_pool(name="scantmp", bufs=2) as stp, \
             tc.tile_pool(name="final", bufs=1) as fin:
            NCHUNK = len(CHUNK_STEPS)
            CHUNK_T0 = [sum(CHUNK_STEPS[:c]) for c in range(NCHUNK)]
            CHMAX = max(CHUNK_STEPS) * W
            A_chunks = []
            B_chunks = []
            FX_chunks = []
            for c in range(NCHUNK):
                CH = CHUNK_STEPS[c] * W
                At = stage.tile([P, CHMAX], f16, tag="Ach", name=f"Ach{c}")[:, :CH]
                Bt = stage.tile([P, CHMAX], f16, tag="Bch", name=f"Bch{c}")[:, :CH]
                Ft = stage.tile([P, CHMAX], f16, tag="Fch", name=f"Fch{c}")[:, :CH]
                sl = slice(CHUNK_T0[c] * W, CHUNK_T0[c] * W + CH)
                nc.sync.dma_start(At[:], A_d[:, sl])
                nc.sync.dma_start(Bt[:], B_d[:, sl])
                nc.sync.dma_start(Ft[:], FX_d[:, sl])
                A_chunks.append(At)
                B_chunks.append(Bt)
                FX_chunks.append(Ft)

            # ---- d-weights (independent of the scan; emitted first so
            # the ACT engine computes them while the scan runs) ----
            KB = K * BINS  # 640
            bt = fin.tile([P, KB], f32, tag="bt")
            bt20 = fin.tile([P, KB], f32, tag="bt20")
            reqb_s = fin.tile([P, KB], f32, tag="reqb_s")
            reqb_p = fin.tile([P, KB], f32, tag="reqb_p")
            bz = fin.tile([P, 4], f32, tag="bz")
            nc.sync.dma_start(bt[:], bt_d[:, :])
            nc.sync.dma_start(bt20[:], bt20_d[:, :])
            nc.sync.dma_start(reqb_s[:], reqb_s_d[:, :])
            nc.sync.dma_start(reqb_p[:], reqb_p_d[:, :])
            nc.sync.dma_start(bz[:], bz_d[:, :])

            nzz = fin.tile([P, 2], f32, tag="nzz")
            nc.gpsimd.tensor_tensor(nzz[:], bz[:, 2:4], bz[:, 2:4], Alu.mult)
            nc.gpsimd.tensor_scalar(nzz[:], nzz[:], -1.0, None, Alu.mult)

            d_tiles = []
            for dim, reqb in ((0, reqb_s), (1, reqb_p)):
                t1 = fin.tile([P, KB], f32, tag=f"t1_{dim}")
                sp = fin.tile([P, KB], f32, tag=f"sp_{dim}")
                dti = fin.tile([P, KB], f32, tag=f"d_{dim}")
                # d = exp(-zeta^2 * ln(1 + exp(beta * (req - s))))
                nc.gpsimd.tensor_tensor(t1[:], reqb[:], bt[:], Alu.subtract)
                nc.scalar.activation(sp[:], t1[:], Act.Exp,
                                     scale=bz[:, dim : dim + 1])
                nc.gpsimd.tensor_scalar(t1[:], sp[:], 1.0, None, Alu.add)
                nc.scalar.activation(sp[:], t1[:], Act.Ln)
                nc.scalar.activation(dti[:], sp[:], Act.Exp,
                                     scale=nzz[:, dim : dim + 1])
                d_tiles.append(dti)

            # ---- the scan (all on DVE, fp16 x20-integer domain) ----
            lo_t = state_pool.tile([P, W], f16, tag="low")
            hi_t = state_pool.tile([P, W], f16, tag="high")
            nc.vector.memset(lo_t[:], 0.0)
            nc.vector.memset(hi_t[:], 20.0)

            t_to_chunk = []
            for c in range(NCHUNK):
                t_to_chunk += [(c, i) for i in range(CHUNK_STEPS[c])]
            for t in range(T):
                tc_idx, tl_idx = t_to_chunk[t]
                A = A_chunks[tc_idx][:, tl_idx * W : (tl_idx + 1) * W]
                B = B_chunks[tc_idx][:, tl_idx * W : (tl_idx + 1) * W]
                FX = FX_chunks[tc_idx][:, tl_idx * W : (tl_idx + 1) * W]
                lo = lo_t[:]
                hi = hi_t[:]
                g1 = stp.tile([P, W], f16, tag="g1")
                Ap = stp.tile([P, W], f16, tag="Ap")
                lo1 = stp.tile([P, W], f16, tag="lo1")
                lo2 = stp.tile([P, W], f16, tag="lo2")
                hs = stp.tile([P, W], f16, tag="hs")
                g2 = stp.tile([P, W], f16, tag="g2")
                Bp = stp.tile([P, W], f16, tag="Bp")
                eq = stp.tile([P, W], f16, tag="eq")
                fxm = stp.tile([P, W], f16, tag="fxm")

                # ordering interleaves the succ/fail sub-chains so
                # dependent ops are not back-to-back (hides the DVE
                # write-ack latency between dependent instructions)
                nc.vector.tensor_tensor(g1[:], A, hi, Alu.is_gt)
                nc.vector.tensor_tensor(g2[:], B, lo, Alu.is_lt)
                nc.vector.tensor_tensor(hs[:], hi, A, Alu.max)
                # succ: Ap = A - 80*(A > high); fail: Bp = B + 80*(B < low)
                nc.vector.scalar_tensor_tensor(
                    Ap[:], g1[:], -80.0, A, Alu.mult, Alu.add)
                nc.vector.scalar_tensor_tensor(
                    Bp[:], g2[:], 80.0, B, Alu.mult, Alu.add)
                nc.vector.tensor_tensor(lo1[:], lo, Ap[:], Alu.max)
                nc.vector.tensor_tensor(hi, hs[:], Bp[:], Alu.min)
                nc.vector.tensor_tensor(lo2[:], lo1[:], B, Alu.min)
                # fixup: low' = lo2 + FX*(lo2 == high')
                nc.vector.tensor_tensor(eq[:], lo2[:], hi, Alu.is_equal)
                nc.vector.tensor_tensor(fxm[:], eq[:], FX, Alu.mult)
                nc.vector.tensor_tensor(lo, lo2[:], fxm[:], Alu.add)

            # ---- final phase (tail after the scan) ----
            # lo/hi are fp16 in the x20 domain; compare against bt20.
            lowf = fin.tile([P, W], f32, tag="lowf")
            highf = fin.tile([P, W], f32, tag="highf")
            nc.vector.tensor_copy(lowf[:], lo_t[:])
            nc.vector.tensor_copy(highf[:], hi_t[:])

            bt3 = bt20[:].rearrange("p (k j) -> p k j", j=BINS)
            UC = []
            for dim in (0, 1):
                lowv = lowf[:, dim * K : (dim + 1) * K]
                highv = highf[:, dim * K : (dim + 1) * K]
                lowb = lowv.unsqueeze(2).broadcast_to((P, K, BINS))
                highb = highv.unsqueeze(2).broadcast_to((P, K, BINS))
                m1 = fin.tile([P, KB], f32, tag=f"m1_{dim}")
                m = fin.tile([P, KB], f32, tag=f"m_{dim}")
                dm = fin.tile([P, KB], f32, tag=f"dm_{dim}")

                nc.vector.tensor_tensor(m1[:].rearrange("p (k j) -> p k j", j=BINS),
                                        bt3, lowb, Alu.is_ge)
                nc.vector.tensor_tensor(m[:].rearrange("p (k j) -> p k j", j=BINS),
                                        bt3, highb, Alu.is_le)
                nc.vector.tensor_tensor(m[:], m[:], m1[:], Alu.mult)
                nc.vector.tensor_tensor(dm[:], d_tiles[dim][:], m[:], Alu.mult)

                U = fin.tile([P, K], f32, tag=f"U_{dim}")
                C = fin.tile([P, K], f32, tag=f"C_{dim}")
                nc.vector.tensor_reduce(
                    U[:], dm[:].rearrange("p (k j) -> p k j", j=BINS),
                    mybir.AxisListType.X, Alu.add)
                nc.vector.tensor_reduce(
                    C[:], m[:].rearrange("p (k j) -> p k j", j=BINS),
                    mybir.AxisListType.X, Alu.add)
                UC.append((U, C))

            # trust = (U0*U1) / (C0*C1)
            uu = fin.tile([P, K], f32, tag="uu")
            cc = fin.tile([P, K], f32, tag="cc")
            rr = fin.tile([P, K], f32, tag="rr")
            tr = fin.tile([P, K], f32, tag="tr")
            nc.vector.tensor_tensor(uu[:], UC[0][0][:], UC[1][0][:], Alu.mult)
            nc.vector.tensor_tensor(cc[:], UC[0][1][:], UC[1][1][:], Alu.mult)
            nc.vector.reciprocal(rr[:], cc[:])
            nc.vector.tensor_tensor(tr[:], uu[:], rr[:], Alu.mult)
            nc.sync.dma_start(out_d[:, :], tr[:])

    _split_sync_waits(nc)
    return nc


def _get_nc():
    if "nc" not in _NC_CACHE:
        _NC_CACHE["nc"] = _build_nc()
    return _NC_CACHE["nc"]


def _marshal_core(inputs, c):
    """Build the per-core input map (slicing/layout/packing, no model math)."""
    n0, n1 = c * NC, (c + 1) * NC

    # caps scaled x20 are odd integers 1..19: exact in fp16.
    obs_s = np.asarray(inputs["obs_task_sens_cap_seq"][:, n0:n1], dtype=np.float32) * np.float32(20.0)
    obs_p = np.asarray(inputs["obs_task_proc_cap_seq"][:, n0:n1], dtype=np.float32) * np.float32(20.0)
    perf = np.asarray(inputs["inptasksperf"][:, n0:n1, :])
    s1 = perf[:, :, 1] != 0   # success bit [T, NC]
    s0 = perf[:, :, 0] != 0   # fail bit

    # A = succ ? cap20 : 0 ; B = fail ? cap20 : 40, for both dims,
    # laid out [P, T*W] with col = t*W + dim*K + k, seq n = p*K + k.
    def lay(x):  # [T, 2, NC] -> [P, T*2*K]
        return np.ascontiguousarray(
            x.reshape(T, 2, P, K).transpose(2, 0, 1, 3).reshape(P, FW))

    zero = np.float32(0.0)
    forty = np.float32(40.0)
    A = np.stack([np.where(s1, obs_s, zero), np.where(s1, obs_p, zero)], axis=1)
    B = np.stack([np.where(s0, obs_s, forty), np.where(s0, obs_p, forty)], axis=1)
    bins_s = np.round(obs_s * 0.5 - 0.5).astype(np.int64)
    bins_p = np.round(obs_p * 0.5 - 0.5).astype(np.int64)
    FX = np.stack([_FIX_OFF[bins_s], _FIX_OFF[bins_p]], axis=1)
    A = lay(A).astype(np.float16)
    B = lay(B).astype(np.float16)
    FX = lay(FX).astype(np.float16)

    def layreq(x):  # [NC] -> [P, K*BINS] broadcast each seq over 10 bins
        r = x.reshape(P, K, 1)
        return np.ascontiguousarray(np.broadcast_to(r, (P, K, BINS)).reshape(P, K * BINS))

    req_s = layreq(np.asarray(inputs["pred_task_sens_cap"][n0:n1, 0], dtype=np.float32))
    req_p = layreq(np.asarray(inputs["pred_task_proc_cap"][n0:n1, 0], dtype=np.float32))
    bt = np.ascontiguousarray(np.broadcast_to(np.tile(STEPS, K), (P, K * BINS))).astype(np.float32)
    st20 = (np.arange(BINS, dtype=np.float32) * 2 + 1).astype(np.float32)  # exact odd ints
    bt20 = np.ascontiguousarray(np.broadcast_to(np.tile(st20, K), (P, K * BINS))).astype(np.float32)
    betas = np.asarray(inputs["betas"], dtype=np.float32)
    zetas = np.asarray(inputs["zetas"], dtype=np.float32)
    bz = np.ascontiguousarray(
        np.broadcast_to(np.concatenate([betas, zetas]).astype(np.float32), (P, 4)))
    return {
        "Aplane": A, "Bplane": B, "FXplane": FX,
        "bt": bt, "bt20": bt20, "reqb_s": req_s, "reqb_p": req_p, "bz": bz,
    }


def kernel(**inputs) -> np.ndarray:
    from concourse.bass_utils import run_bass_kernel_spmd

    nc = _get_nc()
    in_maps = [_marshal_core(inputs, c) for c in range(N_CORES)]
    res = run_bass_kernel_spmd(nc, in_maps, core_ids=list(range(N_CORES)))
    out = np.empty((N_TOTAL, 1), dtype=np.float32)
    for c in range(N_CORES):
        out[c * NC : (c + 1) * NC, 0] = res.results[c]["trust"].reshape(NC)
    return out


# ---------------------------------------------------------------------------
# numpy mirror of the device algorithm (for validation only)
def _numpy_mirror(inputs):
    obs_s = np.asarray(inputs["obs_task_sens_cap_seq"], dtype=np.float32) * 20.0
    obs_p = np.asarray(inputs["obs_task_proc_cap_seq"], dtype=np.float32) * 20.0
    perf = np.asarray(inputs["inptasksperf"])
    p0 = (perf[:, :, 0] != 0)
    p1 = (perf[:, :, 1] != 0)
    betas = np.asarray(inputs["betas"], dtype=np.float32)
    zetas = np.asarray(inputs["zetas"], dtype=np.float32)
    req = [np.asarray(inputs["pred_task_sens_cap"][:, 0], dtype=np.float32),
           np.asarray(inputs["pred_task_proc_cap"][:, 0], dtype=np.float32)]
    N = obs_s.shape[1]
    trust = np.ones(N, dtype=np.float32)
    h = np.float16
    for dim, obs in ((0, obs_s), (1, obs_p)):
        bins = np.round(obs * 0.5 - 0.5).astype(np.int64)
        FXD = _FIX_OFF[bins].astype(h)
        low = np.zeros(N, h)
        high = np.full(N, 20.0, h)
        for t in range(T):
            A = np.where(p1[t], obs[t], np.float32(0.0)).astype(h)
            B = np.where(p0[t], obs[t], np.float32(40.0)).astype(h)
            g1 = (A > high).astype(h)
            hs = np.maximum(high, A)
            Ap = ((np.float16(-80.0)) * g1 + A).astype(h)
            lo1 = np.maximum(low, Ap)
            lo2 = np.minimum(lo1, B)
            g2 = (B < low).astype(h)
            Bp = (np.float16(80.0) * g2 + B).astype(h)
            high = np.minimum(hs, Bp)
            eqm = (lo2 == high).astype(h)
            low = (FXD[t] * eqm + lo2).astype(h)
        lo32 = low.astype(np.float32)
        hi32 = high.astype(np.float32)
        st20 = (np.arange(BINS, dtype=np.float32) * 2 + 1).astype(np.float32)
        m = ((st20[None, :] >= lo32[:, None]) & (st20[None, :] <= hi32[:, None]))
        z2 = np.float32(zetas[dim]) * np.float32(zetas[dim])
        p = np.float32(betas[dim]) * (req[dim][:, None] - STEPS[None, :])
        d = np.exp(-z2 * np.log1p(np.exp(p.astype(np.float64))))
        u = (d * m).sum(1) / m.sum(1)
        trust = trust * u.astype(np.float32)
    return trust[:, None]
